# revision 1
# baseline (speedup 1.0000x reference)
"""Trainium2 Bass kernel for CustomMultiHeadSelfAttention.

Problem shapes: B=4, N=2048, E=1024, H=16, HD=64.

Sharding (8 cores): core c -> batch b = c//2, head-group g = c%2
(heads 8g..8g+7, i.e. feature cols [512g, 512g+512) of q/k/v).
Each core:
  - projects its batch's qkv slice -> Q^T,K^T (head-pair packed, d on
    partitions) and V (natural, seq on partitions)
  - full attention for its 8 heads (exact softmax, no max subtraction --
    scores are ~N(0,1) so exp never overflows)
  - partial out_proj: attnout_local [2048,512] @ W_out[:,cols]^T -> [2048,1024]
Host unshards: y[b] = partial[2b] + partial[2b+1] + out_proj_b.

All matmuls run in bf16 with fp32 PSUM accumulation (host pre-casts the
sharded operands); softmax statistics stay fp32.

v2 structure (vs baseline):
  - S^T matmuls are full 128x128-mode with zero-padded K^T stationaries
    (PAD_S): kills the 64-row tiling-mode entry/exit LDWEIGHTS
    serialization observed in the trace (~200ns per key block).
  - software-pipelined emission: each step emits S+exp for key block k
    and the PV matmuls for key block k-1, so the PE FIFO never blocks
    on the current exp.
  - attention starts as soon as pair-0 K (chunks 0-1), Q chunk 0 and
    V tb0-3 are projected; ALL other projection work is dripped through
    the filler queue between attention steps.
  - HAM pre-warm: junk matmuls during the initial DMA wait.
  - out-projection staged per 128-token block and flushed continuously;
    the final chunk's pair-3 term uses the (by then free) s_ps banks.

PE utilization tricks kept from baseline:
  - PV: two heads as 128-col stationary [V_h | ones] blocks; the ones
    matmul columns produce each head's softmax row-sums for free.
"""

import sys

if "/opt/trn_rl_repo" not in sys.path:
    sys.path.insert(0, "/opt/trn_rl_repo")

from collections import deque
from contextlib import ExitStack

import ml_dtypes
import numpy as np

import concourse.tile as tile
from concourse import bacc, mybir
from concourse.bass_utils import run_bass_kernel_spmd

B, N, E, H = 4, 2048, 1024, 16
HD = E // H          # 64
HL = H // 2          # 8 local heads per core
EL = HL * HD         # 512 local feature cols per core
NP = 128             # partitions
NPAIRS = HL // 2     # 4 head pairs per core (2 heads packed in 128 partitions)
QC = 512             # query chunk (free dim of S^T / PV matmuls)
NQC = N // QC        # 4
NKB = N // NP        # 16 key blocks of 128
TC = 512             # token chunk in projections
EC = E // NP         # 8 contraction chunks in the in-projection

BF16 = mybir.dt.bfloat16
FP32 = mybir.dt.float32

PAD_S = False        # S^T via zero-padded 128-row matmuls (vs 64-row pairs)

_CACHED = {}


def build_kernel():
    """Build the per-core Bass program (identical for every core)."""
    nc = bacc.Bacc("TRN2", target_bir_lowering=False, debug=False, num_devices=8)

    # bf16 operand blobs + one fp32 bias blob, all partition-major
    xt_d = nc.dram_tensor("xt", [NP, EC * N], BF16, kind="ExternalInput").ap()
    wt_d = nc.dram_tensor("wt", [NP, EC * 3 * EL], BF16, kind="ExternalInput").ap()
    wot_d = nc.dram_tensor("wot", [NP, NPAIRS * E], BF16, kind="ExternalInput").ap()
    bias_d = nc.dram_tensor(
        "bias", [NP, 2 * NPAIRS + EL], FP32, kind="ExternalInput"
    ).ap()
    # output, partition-major: y_d[p, tb, j] = y[tb*128 + p, j]
    y_d = nc.dram_tensor("y", [NP, NKB, E], FP32, kind="ExternalOutput").ap()

    with tile.TileContext(nc) as tc:
        _emit(tc, xt_d, wt_d, wot_d, bias_d, y_d)
    nc.compile()
    return nc


def _emit(tc, xt_d, wt_d, wot_d, bias_d, y_d):
    nc = tc.nc
    ctx = ExitStack()
    with ctx:
        singles = ctx.enter_context(tc.tile_pool(name="singles", bufs=1))
        proj_ps = ctx.enter_context(tc.tile_pool(name="proj_ps", bufs=2, space="PSUM"))
        s_ps = ctx.enter_context(tc.tile_pool(name="s_ps", bufs=2, space="PSUM"))
        pv_ps = ctx.enter_context(tc.tile_pool(name="pv_ps", bufs=1, space="PSUM"))
        pv2_ps = ctx.enter_context(tc.tile_pool(name="pv2_ps", bufs=1, space="PSUM"))
        epool = ctx.enter_context(tc.tile_pool(name="epool", bufs=6))
        rpool = ctx.enter_context(tc.tile_pool(name="rpool", bufs=2))
        ypool = ctx.enter_context(tc.tile_pool(name="ypool", bufs=3))

        # ---- resident SBUF tensors -----------------------------------------
        xt4_sb = singles.tile([NP, 4, EC, N // 4], BF16)  # X^T, quarter-major
        wtv_sb = singles.tile([NP, EC, EL], BF16)         # W_v^T
        wt_sb = singles.tile([NP, EC, 2 * EL], BF16)      # W_qk^T
        wot_sb = singles.tile([NP, NPAIRS, E], BF16)      # W_out^T [el, j]
        qt_sb = singles.tile([NP, NPAIRS, N], BF16)       # Q^T (pair-packed)
        kt_sb = singles.tile([NP, NPAIRS, N], BF16)       # K^T (pair-packed)
        # V packed per head as a 128-col stationary block: even head in a
        # pair -> [V_h | ones], odd head -> [ones | V_h].  The ones block
        # makes every PV matmul also produce that head's softmax row-sums,
        # broadcast across 64 partitions, on the half not holding data.
        vo_sb = singles.tile([NP, NKB, HL, NP], BF16)
        at_sb = singles.tile([NP, NPAIRS, N], BF16)       # attnout^T (pair-packed)
        bias_sb = singles.tile([NP, 2 * NPAIRS + EL], FP32)
        warm_sb = singles.tile([NP, NP], BF16)            # HAM pre-warm junk

        xt_dv = xt_d.rearrange("p (tq ec t) -> p tq ec t", tq=4, ec=EC)
        wqk_dv = wt_d[:, EC * EL:].rearrange("p (ec c) -> p ec c", ec=EC)
        # DMA order = first-needed-first: bias, X^T q0, pair-0 Q/K weight
        # columns (so attention's critical path starts early), W_v, the
        # remaining Q/K columns, X^T q1-3, W_out
        nc.sync.dma_start(bias_sb[:], bias_d)
        nc.sync.dma_start(xt4_sb[:, 0], xt_dv[:, 0])
        nc.sync.dma_start(wt_sb[:, :, 0:NP], wqk_dv[:, :, 0:NP])
        nc.sync.dma_start(wt_sb[:, :, EL:EL + NP], wqk_dv[:, :, EL:EL + NP])
        nc.sync.dma_start(
            wtv_sb[:],
            wt_d[:, 0:EC * EL].rearrange("p (ec c) -> p ec c", ec=EC))
        nc.sync.dma_start(xt4_sb[:, 1], xt_dv[:, 1])
        nc.sync.dma_start(wt_sb[:, :, NP:EL], wqk_dv[:, :, NP:EL])
        nc.sync.dma_start(wt_sb[:, :, EL + NP:], wqk_dv[:, :, EL + NP:])
        nc.sync.dma_start(xt4_sb[:, 2], xt_dv[:, 2])
        nc.sync.dma_start(xt4_sb[:, 3], xt_dv[:, 3])
        nc.sync.dma_start(wot_sb[:], wot_d.rearrange("p (pr j) -> p pr j", pr=NPAIRS))
        bqk_sb = bias_sb[:, 0:2 * NPAIRS]
        bv_bc = bias_sb[:, 2 * NPAIRS:]

        # HAM pre-warm: keep the PE active during the initial DMA wait so
        # the clock gate is at 8/8 when real work starts.  No data deps.
        nc.vector.memset(warm_sb[:], 0.0)
        for _ in range(10):
            wps = proj_ps.tile([NP, TC], FP32, tag="ps")
            for r in range(4):
                nc.tensor.matmul(
                    wps[:, r * NP:(r + 1) * NP], lhsT=warm_sb[:], rhs=warm_sb[:],
                    start=True, stop=True,
                )

        # ---- projection jobs ------------------------------------------------
        def v_job(tb):
            nc.vector.memset(vo_sb[:, tb], 1.0)
            ps = proj_ps.tile([NP, EL], FP32, tag="ps")
            for ec in range(EC):
                nc.tensor.matmul(
                    ps[:],
                    lhsT=xt4_sb[:, tb // 4, ec, (tb % 4) * NP:(tb % 4 + 1) * NP],
                    rhs=wtv_sb[:, ec, :],
                    start=(ec == 0),
                    stop=(ec == EC - 1),
                )
            psv = ps[:].rearrange("p (h two d) -> p h two d", two=2, d=HD)
            bvv = bv_bc.rearrange("p (h two d) -> p h two d", two=2, d=HD)
            vov = vo_sb[:, tb].rearrange("p (h two) f -> p h two f", two=2)
            # even heads of each pair -> cols 0:64, odd heads -> cols 64:128
            nc.vector.tensor_tensor(
                vov[:, :, 0, 0:HD], psv[:, :, 0, :], bvv[:, :, 0, :],
                mybir.AluOpType.add,
            )
            nc.vector.tensor_tensor(
                vov[:, :, 1, HD:NP], psv[:, :, 1, :], bvv[:, :, 1, :],
                mybir.AluOpType.add,
            )

        def q_jobs(p, t):
            """Two half-jobs building Q^T tile (pair p, token chunk t)."""
            coff, bcol = p * NP, p
            box = {}

            def half1():
                ps = proj_ps.tile([NP, TC], FP32, tag="ps")
                box["ps"] = ps
                for ec in range(EC // 2):
                    nc.tensor.matmul(
                        ps[:],
                        lhsT=wt_sb[:, ec, coff:coff + NP],
                        rhs=xt4_sb[:, t, ec, :],
                        start=(ec == 0), stop=False,
                    )

            def half2():
                ps = box["ps"]
                for ec in range(EC // 2, EC):
                    nc.tensor.matmul(
                        ps[:],
                        lhsT=wt_sb[:, ec, coff:coff + NP],
                        rhs=xt4_sb[:, t, ec, :],
                        start=False, stop=(ec == EC - 1),
                    )
                nc.vector.tensor_tensor(
                    qt_sb[:, p, t * TC:(t + 1) * TC], ps[:],
                    bqk_sb[:, bcol:bcol + 1].to_broadcast((NP, TC)),
                    mybir.AluOpType.add,
                )
            return [half1, half2]

        def k_jobs(p, t):
            """Two half-jobs building the padded K^T tiles (pair p, chunk t)."""
            coff, bcol = EL + p * NP, NPAIRS + p
            box = {}

            def half1():
                ps = proj_ps.tile([NP, TC], FP32, tag="ps")
                box["ps"] = ps
                for ec in range(EC // 2):
                    nc.tensor.matmul(
                        ps[:],
                        lhsT=wt_sb[:, ec, coff:coff + NP],
                        rhs=xt4_sb[:, t, ec, :],
                        start=(ec == 0), stop=False,
                    )

            def half2():
                ps = box["ps"]
                for ec in range(EC // 2, EC):
                    nc.tensor.matmul(
                        ps[:],
                        lhsT=wt_sb[:, ec, coff:coff + NP],
                        rhs=xt4_sb[:, t, ec, :],
                        start=False, stop=(ec == EC - 1),
                    )
                nc.vector.tensor_tensor(
                    kt_sb[:, p, t * TC:(t + 1) * TC], ps[:],
                    bqk_sb[:, bcol:bcol + 1].to_broadcast((NP, TC)),
                    mybir.AluOpType.add,
                )
            return [half1, half2]

        def outproj_jobs(q):
            """16 half-jobs: out-projection for q-chunk q, staged per tb."""
            jobs = []
            for i, tb in enumerate(range(q * QC // NP, (q + 1) * QC // NP)):
                box = {}
                for jc in range(E // TC):
                    def half1(tb=tb, jc=jc, box=box):
                        ps = proj_ps.tile([NP, TC], FP32, tag="ps")
                        box["ps"] = ps
                        for p in range(NPAIRS // 2):
                            nc.tensor.matmul(
                                ps[:],
                                lhsT=at_sb[:, p, tb * NP:(tb + 1) * NP],
                                rhs=wot_sb[:, p, jc * TC:(jc + 1) * TC],
                                start=(p == 0), stop=False,
                            )

                    def half2(tb=tb, jc=jc, box=box):
                        ps = box["ps"]
                        for p in range(NPAIRS // 2, NPAIRS):
                            nc.tensor.matmul(
                                ps[:],
                                lhsT=at_sb[:, p, tb * NP:(tb + 1) * NP],
                                rhs=wot_sb[:, p, jc * TC:(jc + 1) * TC],
                                start=False, stop=(p == NPAIRS - 1),
                            )
                        if jc == 0:
                            box["ytb"] = ypool.tile([NP, E], FP32, tag="ytb",
                                                    name="ytb")
                        ytb = box["ytb"]
                        nc.vector.tensor_copy(
                            ytb[:, jc * TC:(jc + 1) * TC], ps[:])
                        if jc == E // TC - 1:
                            # gpsimd queue: keeps the sync queue free for
                            # the normalize partition-swap DMAs (a parked
                            # dma_start blocks everything behind it)
                            nc.gpsimd.dma_start(
                                y_d[:, tb:tb + 1, :],
                                ytb[:].rearrange("p (o e) -> p o e", o=1))
                    jobs.append(half1)
                    jobs.append(half2)
            return jobs

        # final q-chunk: pairs 0-2 contracted during the last attention unit
        # into SBUF staging; the pair-3 term runs in the tail on the freed
        # s_ps banks.
        ybF = singles.tile([NP, NQC, E], FP32)

        def outproj3_partial_jobs():
            jobs = []
            for i, tb in enumerate(range((NQC - 1) * QC // NP, NQC * QC // NP)):
                for jc in range(E // TC):
                    def pjob(i=i, tb=tb, jc=jc):
                        ps = proj_ps.tile([NP, TC], FP32, tag="ps")
                        for p in range(NPAIRS - 1):
                            nc.tensor.matmul(
                                ps[:],
                                lhsT=at_sb[:, p, tb * NP:(tb + 1) * NP],
                                rhs=wot_sb[:, p, jc * TC:(jc + 1) * TC],
                                start=(p == 0), stop=(p == NPAIRS - 2),
                            )
                        nc.vector.tensor_copy(
                            ybF[:, i, jc * TC:(jc + 1) * TC], ps[:])
                    jobs.append(pjob)
            return jobs

        def outproj3_final():
            q = NQC - 1
            # one flush per engine queue so the final 2 MiB drains in
            # parallel instead of serializing on one queue
            dma_engs = [nc.sync, nc.gpsimd, nc.scalar, nc.sync]
            for i, tb in enumerate(range(q * QC // NP, (q + 1) * QC // NP)):
                ps = s_ps.tile([NP, 2, QC], FP32, tag="st")
                for jc in range(E // TC):
                    nc.tensor.matmul(
                        ps[:, jc, :],
                        lhsT=at_sb[:, NPAIRS - 1, tb * NP:(tb + 1) * NP],
                        rhs=wot_sb[:, NPAIRS - 1, jc * TC:(jc + 1) * TC],
                        start=True, stop=True,
                    )
                ybs = ybF[:, i, :]
                nc.vector.tensor_tensor(
                    ybs, ybs, ps[:].rearrange("p two q -> p (two q)"),
                    mybir.AluOpType.add)
                dma_engs[i].dma_start(y_d[:, tb:tb + 1, :],
                                      ybF[:, i:i + 1, :])

        # ---- eager prologue -------------------------------------------------
        # Just enough for attention unit (q0, p0) to start: K pair-0
        # chunk 0 (key blocks 0-3) and Q pair-0 chunk 0 first (their
        # weight slices arrive first), then V tb0-3.
        for fn in k_jobs(0, 0) + q_jobs(0, 0):
            fn()
        for tb in range(4):
            v_job(tb)

        # ---- filler queue ---------------------------------------------------
        # Every remaining projection drips between attention steps.
        # Entries are (label, fn); labels gate forced drains before the
        # unit that consumes them.
        jobq = deque()

        def push(label, fns):
            for fn in fns:
                jobq.append((label, fn))

        push("k0", k_jobs(0, 1))      # key blocks 4-7 (needed from step 2)
        push("k0", k_jobs(0, 2))      # key blocks 8-11 (needed from step 4)
        push("v", [lambda tb=4: v_job(4)])
        push("k0", k_jobs(0, 3))      # key blocks 12-15 (needed from step 6)
        push("v", [lambda tb=5: v_job(5)])
        push("q0c1", q_jobs(0, 1))    # needed by unit 1
        push("q0c2", q_jobs(0, 2))    # needed by unit 2
        for tb in range(6, 16):
            push("v", [lambda tb=tb: v_job(tb)])
        push("p0rest", q_jobs(0, 3))

        def pair_jobs(p):
            fns = []
            for t in range(4):
                fns += q_jobs(p, t)
            for t in range(4):
                fns += k_jobs(p, t)
            return fns

        # Unit order: chunks 0-2 interleaved across pairs (so each pair's
        # Q/K projections spread over three units and each chunk's
        # out-projection can start mid-run), chunk 3 last.
        sched = ([(0, 0), (1, 0), (2, 0), (0, 1), (1, 1), (0, 2),
                  (2, 1), (1, 2), (0, 3), (2, 2), (1, 3), (2, 3)]
                 + [(3, p) for p in range(NPAIRS)])

        # labels that must be fully drained before unit ui's S matmuls
        need_by_unit = {
            1: ["k0", "q0c1"],
            2: ["q0c2"],
            3: ["p1"],
            5: ["p2"],
            8: ["p3"],
            15: ["v", "p0rest", "op0", "op1", "op2"],
        }

        def pop_one():
            if jobq:
                jobq.popleft()[1]()

        def drain(labels):
            while jobq and any(lbl in labels for lbl, _ in jobq):
                jobq.popleft()[1]()

        # ---- software-pipelined attention ----------------------------------
        # Per step: fillers + the PV matmuls for the PREVIOUS pair of key
        # blocks (whose exps have had a full step to complete), then S+exp
        # for the current pair -- the PE FIFO never parks on a waiting
        # matmul, and by the time the S visit issues BOTH of its PSUM
        # tiles are free, so all 4 row-group matmuls go out as one
        # 64-row-mode burst.
        pending_pv = [None]
        uctx = {}

        def emit_S(st, q, p, kb):
            qs = slice(q * QC, (q + 1) * QC)
            ks = slice(kb * NP, (kb + 1) * NP)
            nc.tensor.matmul(
                st[:, 0, :],
                lhsT=kt_sb[0:HD, p, ks], rhs=qt_sb[0:HD, p, qs],
                start=True, stop=True,
            )
            nc.tensor.matmul(
                st[:, 1, :],
                lhsT=kt_sb[HD:NP, p, ks], rhs=qt_sb[HD:NP, p, qs],
                start=True, stop=True,
            )

        def normalize(q, p, last=False):
            qs = slice(q * QC, (q + 1) * QC)
            pvA, pvB = uctx.pop("pv")
            if last:
                # tail: nothing needs the PV banks next, so skip the
                # evacuation copies and read PSUM directly
                cA, cB = pvA, pvB
            else:
                # evacuate both PV banks to SBUF immediately so the next
                # unit's matmuls can reuse them; the chain below runs off
                # the PE critical path
                cA = rpool.tile([NP, QC], FP32, tag="cA")
                cB = rpool.tile([NP, QC], FP32, tag="cB")
                nc.vector.tensor_copy(cA[:], pvA[:])
                nc.vector.tensor_copy(cB[:], pvB[:])
            rcA = rpool.tile([NP, QC], FP32, tag="rcA", bufs=1)
            rcB = rpool.tile([NP, QC], FP32, tag="rcB", bufs=1)
            rc2 = rpool.tile([NP, QC], FP32, tag="rc2", bufs=1)
            # full-tile reciprocals (the unused data halves produce junk
            # that is never read); custom DVE ops run at partition base 0
            nc.vector.reciprocal_approx_fast(rcA[:], cA[:])
            nc.vector.reciprocal_approx_fast(rcB[:], cB[:])
            # move each head's 1/sum onto its data partitions
            nc.sync.dma_start(rc2[0:HD, :], rcA[HD:NP, :])
            nc.sync.dma_start(rc2[HD:NP, :], rcB[0:HD, :])
            nc.vector.tensor_mul(at_sb[0:HD, p, qs], cA[0:HD, :],
                                 rc2[0:HD, :])
            nc.vector.tensor_mul(at_sb[HD:NP, p, qs], cB[HD:NP, :],
                                 rc2[HD:NP, :])

        def make_pv(et0, et1, q, p, g2):
            first, last = (g2 == 0), (g2 == NKB // 2 - 1)

            def pv():
                if first:
                    uctx["pv"] = (
                        pv_ps.tile([NP, QC], FP32, tag="pv", name="pvA"),
                        pv2_ps.tile([NP, QC], FP32, tag="pv2", name="pvB"),
                    )
                pvA, pvB = uctx["pv"]
                # fused PV+rowsum: full 128-col stationary operand
                # pvA = [dataA | sumsA], pvB = [sumsB | dataB]
                for j, et in ((0, et0), (1, et1)):
                    kb = 2 * g2 + j
                    st_acc = first and j == 0
                    sp_acc = last and j == 1
                    nc.tensor.matmul(
                        pvA[:],
                        lhsT=vo_sb[:, kb, 2 * p, :],
                        rhs=et[:, 0, :], start=st_acc, stop=sp_acc,
                    )
                    nc.tensor.matmul(
                        pvB[:],
                        lhsT=vo_sb[:, kb, 2 * p + 1, :],
                        rhs=et[:, 1, :], start=st_acc, stop=sp_acc,
                    )
                if last:
                    normalize(q, p, last=(q == NQC - 1 and p == NPAIRS - 1))
            return pv

        for ui, (q, p) in enumerate(sched):
            if ui in need_by_unit:
                drain(need_by_unit[ui])
            if ui == 1:
                push("p1", pair_jobs(1))
            elif ui == 2:
                push("p2", pair_jobs(2))
            elif ui == 5:
                push("p3", pair_jobs(3))
            elif ui == 9:
                push("op0", outproj_jobs(0))
            elif ui == 11:
                push("op1", outproj_jobs(1))
            elif ui == 12:
                push("op2", outproj_jobs(2))
            elif ui == 15:
                push("op3", outproj3_partial_jobs())
            for g2 in range(NKB // 2):
                # At a unit boundary the lagged flush carries the previous
                # unit's normalize, and a popped filler may read the
                # attnout it writes (program order defines dependencies):
                # flush first there.  Mid-unit, pop first so fillers
                # stream while the previous exps finish.
                npop = (3 if ui == 0 else
                        2 if ui <= 2 else
                        1 if ui <= 9 or ui == 15 else
                        2 if g2 % 2 == 0 else 1)
                if g2 == 0:
                    if pending_pv[0] is not None:
                        pending_pv[0]()
                        pending_pv[0] = None
                    for _ in range(npop):
                        pop_one()
                else:
                    for _ in range(npop):
                        pop_one()
                    if pending_pv[0] is not None:
                        pending_pv[0]()
                        pending_pv[0] = None
                st0 = s_ps.tile([NP, 2, QC], FP32, tag="st")
                st1 = s_ps.tile([NP, 2, QC], FP32, tag="st")
                et0 = epool.tile([NP, 2, QC], BF16, tag="et")
                et1 = epool.tile([NP, 2, QC], BF16, tag="et")
                emit_S(st0, q, p, 2 * g2)
                emit_S(st1, q, p, 2 * g2 + 1)
                # exp with the 1/sqrt(HD) score scale fused in
                nc.scalar.activation(
                    et0[:], st0[:], mybir.ActivationFunctionType.Exp,
                    scale=0.125,
                )
                nc.scalar.activation(
                    et1[:], st1[:], mybir.ActivationFunctionType.Exp,
                    scale=0.125,
                )
                pending_pv[0] = make_pv(et0, et1, q, p, g2)

        # drain: last key block's PV + normalize, leftovers, final term
        pending_pv[0]()
        while jobq:
            jobq.popleft()[1]()
        outproj3_final()


def shard_inputs(qkv, in_proj_w, in_proj_b, out_proj_w):
    """Build the 8 per-core input maps (host-side transpose + bf16 cast).

    All device tensors are partition-major [128, free] so each DMA run is
    long and contiguous.
    """
    bf = ml_dtypes.bfloat16
    in_maps = []
    for c in range(8):
        b, g = c // 2, c % 2
        cs = slice(g * EL, (g + 1) * EL)
        # X^T [E, N] -> [p, tq, ec, 512] (quarter-major so each quarter is
        # one long contiguous DMA run per partition)
        xt = np.ascontiguousarray(
            qkv[b].T.reshape(EC, NP, 4, N // 4).transpose(1, 2, 0, 3)
            .reshape(NP, EC * N)
        ).astype(bf)
        w_l = np.concatenate(
            [in_proj_w[cs], in_proj_w[E:2 * E][cs]], 0
        )  # [2*EL, E] q|k
        wv_l = in_proj_w[2 * E:3 * E][cs]  # [EL, E]
        # v-section first (contiguous), then q|k section
        wtv = wv_l.T.reshape(EC, NP, EL).transpose(1, 0, 2).reshape(NP, -1)
        wtqk = w_l.T.reshape(EC, NP, 2 * EL).transpose(1, 0, 2).reshape(NP, -1)
        wt = np.ascontiguousarray(
            np.concatenate([wtv, wtqk], axis=1)
        ).astype(bf)
        wot = np.ascontiguousarray(
            out_proj_w[:, cs].T.reshape(NPAIRS, NP, E).transpose(1, 0, 2)
            .reshape(NP, -1)
        ).astype(bf)
        bias = np.empty((NP, 2 * NPAIRS + EL), np.float32)
        bq = in_proj_b[cs]
        bk = in_proj_b[E:2 * E][cs]
        for p in range(NPAIRS):
            bias[:, p] = bq[p * NP:(p + 1) * NP]
            bias[:, NPAIRS + p] = bk[p * NP:(p + 1) * NP]
        bias[:, 2 * NPAIRS:] = in_proj_b[2 * E:3 * E][cs][None, :]
        in_maps.append({"xt": xt, "wt": wt, "wot": wot, "bias": bias})
    return in_maps


def unshard_output(ys, out_proj_b):
    # ys[c] is [128, 16, 1024] partition-major: y[tb*128+p, j] = ys[p, tb, j]
    full = [np.asarray(y).transpose(1, 0, 2).reshape(N, E) for y in ys]
    out = np.stack([full[2 * b] + full[2 * b + 1] for b in range(B)])
    out += out_proj_b[None, None, :]
    return out.astype(np.float32)


def kernel(qkv, in_proj_w, in_proj_b, out_proj_w, out_proj_b):
    qkv = np.asarray(qkv, np.float32)
    in_proj_w = np.asarray(in_proj_w, np.float32)
    in_proj_b = np.asarray(in_proj_b, np.float32)
    out_proj_w = np.asarray(out_proj_w, np.float32)
    out_proj_b = np.asarray(out_proj_b, np.float32)

    if "nc" not in _CACHED:
        _CACHED["nc"] = build_kernel()
    nc = _CACHED["nc"]

    in_maps = shard_inputs(qkv, in_proj_w, in_proj_b, out_proj_w)
    res = run_bass_kernel_spmd(nc, in_maps, core_ids=list(range(8)))
    ys = [res.results[c]["y"] for c in range(8)]
    return unshard_output(ys, out_proj_b)

